# revision 1
# baseline (speedup 1.0000x reference)
"""BatchHardTripletLoss on 8 Trainium2 NeuronCores.

Math (on rows sorted by label):
  e = embeddings / ||embeddings||          (row L2 norm)
  S = e @ e.T                              (cosine similarity Gram matrix)
  T = S - 4 * [label_i == label_j]
  loss_row = relu(max_j T - min_j T - 3.7)  (= relu(hard_pos - hard_neg + 0.3))
  out = mean(loss_row)

min_j T always lands on a same-label element (the -4 shift beats any s >= -1);
self (s=1) is never the min unless the row has no other positive, in which
case max_j T < 0.7 keeps the relu at zero either way (verified: global max
non-same s = 0.304 for this input family).

Sharding: rows sorted by label, grouped into 64 tiles of 128 rows. Core c
owns global row-tiles g = 8m + c (m = 0..7, interleaved). With sorted labels,
all positives of row-tile g live in columns [128g - Cmax, 128g + 128 + Cmax);
for every core the m-th tile's positive window is inside the *same* column
window W(m) = [1024m - 128, 1024m + 1280), so one SPMD program serves all
cores: the eq-label mask + min-mining runs only on W(m), plain max mining on
the rest. Requires max label multiplicity <= 129 (checked at runtime).

Layout: the host ships the embeddings both natural ([N, D], for row norms)
and transposed ([D, N], the matmul operand). The device computes
r = 1/||row|| in natural layout, round-trips r through DRAM to get it
replicated across partitions, and column-scales the transposed operand
in place on GpSimd. No on-device transposes (the DMA xbar transpose
serializes on the Sync engine at ~1.2 us per 128x128 chunk).
"""

import numpy as np
from contextlib import ExitStack

N, D = 8192, 512
NCORES = 8
M_TILES = 8          # row tiles per core
K_TILES = D // 128   # 4
NQ = 4               # column quads of 2048
QW = 2048
MARGIN_C = 3.7       # 4 - 1 + MARGIN(0.3); loss = relu(maxT - minT - 3.7)


def _window(m):
    """Column window [lo, hi) containing every positive of row-tile m on
    every core (global tiles g = 8m + c, c in 0..7)."""
    lo = max(0, 1024 * m - 128)
    hi = min(N, 1024 * m + 1024 + 256)
    return lo, hi


def _pieces():
    """piece_table[(q, m)] = [(lo, hi, is_window, slot)] with slot ids
    assigned globally per m across quads."""
    table = {}
    for m in range(M_TILES):
        wlo, whi = _window(m)
        nslot = 0
        wslot = 0
        for q in range(NQ):
            qlo, qhi = q * QW, (q + 1) * QW
            a, b = max(qlo, wlo), min(qhi, whi)
            pieces = []
            if a >= b:
                pieces.append((qlo, qhi, False, nslot))
                nslot += 1
            else:
                if qlo < a:
                    pieces.append((qlo, a, False, nslot))
                    nslot += 1
                pieces.append((a, b, True, (nslot, wslot)))
                nslot += 1
                wslot += 1
                if b < qhi:
                    pieces.append((b, qhi, False, nslot))
                    nslot += 1
            table[(q, m)] = pieces
        assert nslot <= 6 and wslot <= 2, (m, nslot, wslot)
    return table


def _build_program():
    import concourse.bass as bass
    import concourse.bacc as bacc
    import concourse.tile as tile
    from concourse import mybir

    f16 = mybir.dt.float16
    f32 = mybir.dt.float32
    Alu = mybir.AluOpType
    Act = mybir.ActivationFunctionType
    Ax = mybir.AxisListType

    nc = bacc.Bacc("TRN2", target_bir_lowering=False, debug=False,
                   num_devices=NCORES)

    embT = nc.dram_tensor("embT", [D, N], f16, kind="ExternalInput").ap()
    emb = nc.dram_tensor("emb", [N, D], f16, kind="ExternalInput").ap()
    blkT = nc.dram_tensor("blkT", [128, K_TILES * 1024], f16,
                          kind="ExternalInput").ap()
    blkn = nc.dram_tensor("blkn", [128 * M_TILES, D], f16,
                          kind="ExternalInput").ap()
    labs = nc.dram_tensor("labs", [N], f16, kind="ExternalInput").ap()
    blklab = nc.dram_tensor("blklab", [128 * M_TILES], f32,
                            kind="ExternalInput").ap()
    out = nc.dram_tensor("out", [1, 1], f32, kind="ExternalOutput").ap()
    # DRAM scratch for the norm round-trip
    rall_d = nc.dram_tensor("rall_d", [N], f32).ap()
    rblk_d = nc.dram_tensor("rblk_d", [128 * M_TILES], f32).ap()

    NEG = -1.0e30
    POS = 1.0e30
    ptab = _pieces()

    with TileCtx(nc, tile) as (tc, ctx):
        persist = ctx.enter_context(tc.tile_pool(name="persist", bufs=1))
        natp = ctx.enter_context(tc.tile_pool(name="nat", bufs=1))
        psum = ctx.enter_context(tc.tile_pool(name="ps", bufs=2, space="PSUM"))
        eqp = ctx.enter_context(tc.tile_pool(name="eq", bufs=2))
        twp = ctx.enter_context(tc.tile_pool(name="tw", bufs=2))

        labels_sb = persist.tile([128, N], f16, tag="labels")
        blklab_sb = persist.tile([128, M_TILES], f32, tag="blklab")
        # ET[k][g]: [128, 2048] fp16 — embT rows k*128..(k+1)*128, col group g
        ET = [[persist.tile([128, QW], f16, tag=f"et{k}_{g}",
                            name=f"et{k}_{g}") for g in range(NQ)]
              for k in range(K_TILES)]
        BlkT = persist.tile([128, K_TILES * 1024], f16, tag="blkt")
        Rg = [persist.tile([128, QW], f32, tag=f"rg{g}", name=f"rg{g}")
              for g in range(NQ)]
        Rblk = persist.tile([128, 1024], f32, tag="rblk2")
        ss_blk = persist.tile([128, M_TILES], f32, tag="ssblk")
        r_blk = persist.tile([128, M_TILES], f32, tag="rblk")
        ss_all = persist.tile([128, 64], f32, tag="ssall")
        r_all = persist.tile([128, 64], f32, tag="rall")
        maxp = persist.tile([128, M_TILES * 6], f32, tag="maxp")
        minp = persist.tile([128, M_TILES * 2], f32, tag="minp")
        maxT = persist.tile([128, M_TILES], f32, tag="maxT")
        minT = persist.tile([128, M_TILES], f32, tag="minT")
        diffs = persist.tile([128, M_TILES], f32, tag="diffs")
        relu_d = persist.tile([128, M_TILES], f32, tag="relud")
        row_loss = persist.tile([128, 1], f32, tag="rowloss")
        ones_sb = persist.tile([128, 1], f32, tag="ones")
        negm = persist.tile([128, 1], f32, tag="negm")
        out_sb = persist.tile([1, 1], f32, tag="outsb")
        sqdump = persist.tile([128, D], f16, tag="sqdump")

        nc.vector.memset(maxp[:], NEG)
        nc.vector.memset(minp[:], POS)
        nc.vector.memset(ones_sb[:], 1.0)
        nc.vector.memset(negm[:], -MARGIN_C)

        # ---------------- block: norms + scale ----------------
        nc.sync.dma_start(out=BlkT[:], in_=blkT)
        for t in range(M_TILES):
            bn = natp.tile([128, D], f16, tag=f"bnat{t}", name=f"bnat{t}")
            nc.sync.dma_start(out=bn[:], in_=blkn[t * 128:(t + 1) * 128, :])
            nc.scalar.activation(sqdump[:], bn[:], Act.Square,
                                 accum_out=ss_blk[:, t:t + 1])
        nc.scalar.activation(r_blk[:], ss_blk[:], Act.Sqrt)
        nc.vector.reciprocal(r_blk[:], r_blk[:])
        nc.sync.dma_start(out=rblk_d.rearrange("(t p) -> p t", p=128),
                          in_=r_blk[:])
        rblk_b = bass.AP(rblk_d.tensor, rblk_d.offset, [[0, 128], [1, 1024]])
        nc.sync.dma_start(out=Rblk[:], in_=rblk_b)
        for k in range(K_TILES):
            nc.gpsimd.tensor_tensor(
                out=BlkT[:, k * 1024:(k + 1) * 1024],
                in0=BlkT[:, k * 1024:(k + 1) * 1024],
                in1=Rblk[:], op=Alu.mult)

        # ---------------- full matrix: per column-group norm + scale ------
        for g in range(NQ):
            ts0 = 16 * g
            for k in range(K_TILES):
                nc.sync.dma_start(
                    out=ET[k][g][:],
                    in_=embT[k * 128:(k + 1) * 128, g * QW:(g + 1) * QW])
            for t in range(ts0, ts0 + 16):
                nt = natp.tile([128, D], f16, tag=f"nat{t % 16}",
                               name=f"nat{g}_{t % 16}")
                nc.sync.dma_start(out=nt[:],
                                  in_=emb[t * 128:(t + 1) * 128, :])
                nc.scalar.activation(sqdump[:], nt[:], Act.Square,
                                     accum_out=ss_all[:, t:t + 1])
            nc.scalar.activation(r_all[:, ts0:ts0 + 16],
                                 ss_all[:, ts0:ts0 + 16], Act.Sqrt)
            nc.vector.reciprocal(r_all[:, ts0:ts0 + 16],
                                 r_all[:, ts0:ts0 + 16])
            seg = rall_d[g * QW:(g + 1) * QW]
            nc.sync.dma_start(out=seg.rearrange("(t p) -> p t", p=128),
                              in_=r_all[:, ts0:ts0 + 16])
            rall_b = bass.AP(seg.tensor, seg.offset, [[0, 128], [1, QW]])
            nc.sync.dma_start(out=Rg[g][:], in_=rall_b)
            for k in range(K_TILES):
                nc.gpsimd.tensor_tensor(out=ET[k][g][:], in0=ET[k][g][:],
                                        in1=Rg[g][:], op=Alu.mult)

        # ---------------- labels (needed only once mining begins) --------
        labs_b = bass.AP(labs.tensor, labs.offset, [[0, 128], [1, N]])
        nc.sync.dma_start(out=labels_sb[:], in_=labs_b)
        nc.sync.dma_start(out=blklab_sb[:],
                          in_=blklab.rearrange("(m p) -> p m", p=128))

        # ---------------- mining ----------------
        for q in range(NQ):
            for m in range(M_TILES):
                ps = psum.tile([128, QW], f32, tag="ps")
                for k in range(K_TILES):
                    lhsT = BlkT[:, k * 1024 + m * 128:k * 1024 + (m + 1) * 128]
                    for j in range(4):
                        nc.tensor.matmul(
                            ps[:, j * 512:(j + 1) * 512],
                            lhsT=lhsT,
                            rhs=ET[k][q][:, j * 512:(j + 1) * 512],
                            start=(k == 0), stop=(k == K_TILES - 1))

                qlo = q * QW
                for (lo, hi, isw, slot) in ptab[(q, m)]:
                    w = hi - lo
                    pslice = ps[:, lo - qlo:hi - qlo]
                    if not isw:
                        nc.vector.tensor_reduce(
                            out=maxp[:, m * 6 + slot:m * 6 + slot + 1],
                            in_=pslice, axis=Ax.X, op=Alu.max)
                    else:
                        nslot, wslot = slot
                        eq4 = eqp.tile([128, 1280], f32, tag="eq4")
                        nc.vector.tensor_scalar(
                            out=eq4[:, :w], in0=labels_sb[:, lo:hi],
                            scalar1=blklab_sb[:, m:m + 1], scalar2=4.0,
                            op0=Alu.is_equal, op1=Alu.mult)
                        tw = twp.tile([128, 1280], f32, tag="tw")
                        nc.vector.tensor_tensor(
                            out=tw[:, :w], in0=pslice, in1=eq4[:, :w],
                            op=Alu.subtract)
                        nc.vector.tensor_reduce(
                            out=maxp[:, m * 6 + nslot:m * 6 + nslot + 1],
                            in_=tw[:, :w], axis=Ax.X, op=Alu.max)
                        nc.vector.tensor_reduce(
                            out=minp[:, m * 2 + wslot:m * 2 + wslot + 1],
                            in_=tw[:, :w], axis=Ax.X, op=Alu.min)

        # ---------------- finale ----------------
        for m in range(M_TILES):
            nc.vector.tensor_reduce(out=maxT[:, m:m + 1],
                                    in_=maxp[:, m * 6:(m + 1) * 6],
                                    axis=Ax.X, op=Alu.max)
            nc.vector.tensor_reduce(out=minT[:, m:m + 1],
                                    in_=minp[:, m * 2:(m + 1) * 2],
                                    axis=Ax.X, op=Alu.min)
        nc.vector.tensor_tensor(out=diffs[:], in0=maxT[:], in1=minT[:],
                                op=Alu.subtract)
        nc.scalar.activation(relu_d[:], diffs[:], Act.Relu, bias=negm[:],
                             accum_out=row_loss[:])
        ps1 = psum.tile([1, 1], f32, tag="ps")
        nc.tensor.matmul(ps1[:], lhsT=row_loss[:], rhs=ones_sb[:],
                         start=True, stop=True)
        nc.scalar.copy(out_sb[:], ps1[:])
        nc.sync.dma_start(out=out, in_=out_sb[:])

    nc.compile()
    return nc


class TileCtx:
    """contextmanager pairing TileContext with an ExitStack (pools close
    before the TileContext schedules)."""

    def __init__(self, nc, tile_mod):
        self.nc = nc
        self.tile_mod = tile_mod

    def __enter__(self):
        self.ctx = ExitStack()
        self.ctx.__enter__()
        self.tc = self.tile_mod.TileContext(self.nc)
        self.tc.__enter__()
        return self.tc, self.ctx

    def __exit__(self, *exc):
        self.ctx.__exit__(*exc)
        return self.tc.__exit__(*exc)


def _prep_inputs(embeddings, labels):
    E = np.ascontiguousarray(np.asarray(embeddings, dtype=np.float32))
    lab = np.asarray(labels).reshape(-1)
    assert E.shape == (N, D)

    order = np.argsort(lab, kind="stable")
    E_s = E[order]
    lab_s = lab[order].astype(np.int64)
    assert np.bincount(lab_s).max() <= 129, "label multiplicity > 129"

    E16 = E_s.astype(np.float16)
    lab16 = lab_s.astype(np.float16)
    embT16 = np.ascontiguousarray(E16.T)

    tiles = E16.reshape(64, 128, D)
    labt = lab16.reshape(64, 128)
    in_maps = []
    for c in range(NCORES):
        gsel = [8 * m + c for m in range(M_TILES)]
        blk = np.ascontiguousarray(tiles[gsel].reshape(128 * M_TILES, D))
        # blkT[p, k*1024 + j] = blk[j, k*128 + p]
        blkT = np.ascontiguousarray(
            blk.reshape(1024, K_TILES, 128).transpose(2, 1, 0)
            .reshape(128, K_TILES * 1024))
        in_maps.append({
            "embT": embT16,
            "emb": E16,
            "blkT": blkT,
            "blkn": blk,
            "labs": lab16,
            "blklab": np.ascontiguousarray(
                labt[gsel].reshape(-1).astype(np.float32)),
        })
    return in_maps


def kernel(embeddings, labels):
    from concourse.bass_utils import run_bass_kernel_spmd

    in_maps = _prep_inputs(embeddings, labels)
    nc = _build_program()
    res = run_bass_kernel_spmd(nc, in_maps, core_ids=list(range(NCORES)))
    global LAST_RESULTS
    LAST_RESULTS = res
    total = sum(float(r["out"][0, 0]) for r in res.results)
    return np.float32(total / N)


LAST_RESULTS = None



# revision 17
# speedup vs baseline: 2.5981x; 2.5981x over previous
"""BatchHardTripletLoss on 8 Trainium2 NeuronCores.

Math (rows sorted by label):
  e = embeddings / ||embeddings||          (host, fp64)
  S = e @ e.T                              (device, fp8e4m3 DoubleRow matmul)
  T = S - 4 * [label_i == label_j]
  loss_row = relu(max_j T - min_j T - 3.7)  (= relu(hard_pos - hard_neg + 0.3))
  out = mean(loss_row)

min_j T always lands on a same-label element (the -4 shift beats any s >= -1);
self (s=1) is never the min unless the row has no other positive, in which
case max_j T < 0.7 keeps the relu at zero either way.

Sharding: rows sorted by label, 64 row-tiles of 128. Core c owns global
tiles g = 8m + c (m = 0..7). With sorted labels all positives of tile g
live within +-(mult-1) rows of the tile, a span <= 256 when the max label
multiplicity is <= 64. The host applies a PER-CORE column permutation to
the gathered/normalized embedding matrix so that tile m's positive window
lands at a FIXED slot: columns [2048*(m//2) + 1536, +1792) for even m,
[2048*(m//2) + 1792, +2048) for odd m. A column permutation doesn't change
row-wise max/min, so mining is: plain max over non-slot pieces, eq-masked
max/min (mask precomputed on host) over the 256-wide slot. One SPMD
program serves all cores; all data-dependent layout lives on the host.

Device pipeline per (quad q, tile m): 8 fp8 DoubleRow matmuls accumulate a
[128, 2048] f32 psum block of S. Plain pieces are max-folded to a scalar;
PSUM is only readable by the Scalar(Act) and Vector engines (GpSimd can
neither touch PSUM nor run max), so most tiles go Act psum->fp16 stage +
DVE 2x-rate fp16 folds, with every 6th tile a direct DVE psum reduce to
balance the two engines. The slot piece is one tensor_tensor_reduce
(T = S - eqmask, accum max) plus a min reduce. maxT/minT -> relu -> sum ->
one scalar DMA per core; the host adds the 8 scalars and divides by N.
"""

import os
import numpy as np
from contextlib import ExitStack

N, D = 8192, 512
NCORES = 8
M_TILES = 8          # row tiles per core
NQ = 4               # column quads
QW = 2048            # quad width
W = 256              # positive-window slot width
MARGIN_C = 3.7       # 4 - 1 + MARGIN(0.3); loss = relu(maxT - minT - 3.7)
NEG = -1.0e30

USE_FP8 = os.environ.get("KFP8", "1") == "1"   # fp8e4m3 + DoubleRow matmuls
USE_DR = os.environ.get("KDR", "1") == "1"     # DoubleRow perf mode


def _slot_rel(m):
    """Slot of tile m lives in quad m//2 at this relative offset."""
    return 1536 if m % 2 == 0 else 1792


def _build_program():
    import concourse.bass as bass
    import concourse.bacc as bacc
    import concourse.tile as tile
    from concourse import mybir

    f16 = mybir.dt.float16
    f32 = mybir.dt.float32
    f8 = mybir.dt.float8e4
    edt = f8 if USE_FP8 else f16
    Alu = mybir.AluOpType
    Act = mybir.ActivationFunctionType
    Ax = mybir.AxisListType

    nc = bacc.Bacc("TRN2", target_bir_lowering=False, debug=False,
                   num_devices=NCORES)

    # ET[p, g*8192 + kp*4096 + ks*2048 + c] = e8perm[g*2048+c, (2kp+ks)*128+p]
    ET = nc.dram_tensor("ET", [128, NQ * 8192], edt, kind="ExternalInput").ap()
    # BLKT[p, kp*2048 + m*256 + ks*128 + r] = blk[m*128+r, (2kp+ks)*128+p]
    BLKT = nc.dram_tensor("BLKT", [128, 4096], edt, kind="ExternalInput").ap()
    # EQM[p, m*256 + j] = 4.0 if label(window col j of m) == label(row p of m)
    EQM = nc.dram_tensor("EQM", [128, M_TILES * W], f16,
                         kind="ExternalInput").ap()
    out = nc.dram_tensor("out", [1, 1], f32, kind="ExternalOutput").ap()

    with TileCtx(nc, tile) as (tc, ctx):
        persist = ctx.enter_context(tc.tile_pool(name="persist", bufs=1))
        psum = ctx.enter_context(tc.tile_pool(name="ps", bufs=2, space="PSUM"))
        stagep = ctx.enter_context(tc.tile_pool(name="stage", bufs=3))
        foldp = ctx.enter_context(tc.tile_pool(name="fold", bufs=3))
        twp = ctx.enter_context(tc.tile_pool(name="tw", bufs=2))

        et = [persist.tile([128, 8192], edt, tag=f"et{g}", name=f"et{g}")
              for g in range(NQ)]
        blkt = persist.tile([128, 4096], edt, tag="blkt")
        eqm = persist.tile([128, M_TILES * W], f16, tag="eqm")
        # per-m partial maxes: 3 clean quads + (1 or 2) slot-quad pieces +
        # slot itself -> 6 slots, padded with NEG
        maxp = persist.tile([128, M_TILES * 6], f32, tag="maxp")
        maxT = persist.tile([128, M_TILES], f32, tag="maxT")
        minT = persist.tile([128, M_TILES], f32, tag="minT")
        diffs = persist.tile([128, M_TILES], f32, tag="diffs")
        relu_d = persist.tile([128, M_TILES], f32, tag="relud")
        row_loss = persist.tile([128, 1], f32, tag="rowloss")
        ones_sb = persist.tile([128, 1], f32, tag="ones")
        negm = persist.tile([128, 1], f32, tag="negm")
        out_sb = persist.tile([1, 1], f32, tag="outsb")

        nc.sync.dma_start(out=blkt[:], in_=BLKT)
        for g in range(NQ):
            nc.sync.dma_start(out=et[g][:],
                              in_=ET[:, g * 8192:(g + 1) * 8192])
        nc.sync.dma_start(out=eqm[:], in_=EQM)

        nc.vector.memset(maxp[:], NEG)
        nc.vector.memset(ones_sb[:], 1.0)
        nc.vector.memset(negm[:], -MARGIN_C)

        evac_ctr = [0]

        def evac(ps, lo, w, out_slot):
            """Max-reduce ps[:, lo:lo+w] (f32 psum) into out_slot (f32).

            PSUM is only readable by Act and DVE; DVE tensor_tensor takes at
            most one PSUM input, and GpSimd's TensorTensor can't do max at
            all. Two routes, ~27:5 mix:
              A: Act stages psum->fp16 SBUF, DVE folds at 2x fp16 rate plus
                 a short reduce.
              V: single DVE tensor_reduce straight off psum (1x f32 rate).
            """
            half, q4, q8 = w // 2, w // 4, w // 8
            route_a = evac_ctr[0] % 6 != 5
            evac_ctr[0] += 1
            if not route_a:
                nc.vector.tensor_reduce(out=out_slot, in_=ps[:, lo:lo + w],
                                        axis=Ax.X, op=Alu.max)
                return
            st = stagep.tile([128, 2048], f16, tag="st")
            nc.scalar.copy(st[:, :w], ps[:, lo:lo + w])
            f1 = foldp.tile([128, 1024], f16, tag="f1")
            nc.vector.tensor_tensor(out=f1[:, :half], in0=st[:, :half],
                                    in1=st[:, half:w], op=Alu.max)
            f2 = foldp.tile([128, 512], f16, tag="f2")
            nc.vector.tensor_tensor(out=f2[:, :q4], in0=f1[:, :q4],
                                    in1=f1[:, q4:half], op=Alu.max)
            f3 = foldp.tile([128, 256], f16, tag="f3")
            nc.vector.tensor_tensor(out=f3[:, :q8], in0=f2[:, :q8],
                                    in1=f2[:, q8:q4], op=Alu.max)
            nc.vector.tensor_reduce(out=out_slot, in_=f3[:, :q8],
                                    axis=Ax.X, op=Alu.max)

        # ---------------- mining ----------------
        for q in range(NQ):
            for m in range(M_TILES):
                ps = psum.tile([128, QW], f32, tag="ps")
                for kp in range(2):
                    if USE_FP8 and USE_DR:
                        lhsT = blkt[:, kp * 2048 + m * 256:
                                    kp * 2048 + (m + 1) * 256].rearrange(
                                        "p (s r) -> p s r", s=2)
                        rhs4 = et[q][:, kp * 4096:(kp + 1) * 4096].rearrange(
                            "p (s c) -> p s c", s=2)
                        for j in range(4):
                            nc.tensor.matmul(
                                ps[:, j * 512:(j + 1) * 512],
                                lhsT=lhsT,
                                rhs=rhs4[:, :, j * 512:(j + 1) * 512],
                                start=(kp == 0), stop=(kp == 1),
                                perf_mode=mybir.MatmulPerfMode.DoubleRow)
                    else:
                        for ks in range(2):
                            lhsT = blkt[:, kp * 2048 + m * 256 + ks * 128:
                                        kp * 2048 + m * 256 + (ks + 1) * 128]
                            base = kp * 4096 + ks * 2048
                            for j in range(4):
                                nc.tensor.matmul(
                                    ps[:, j * 512:(j + 1) * 512],
                                    lhsT=lhsT,
                                    rhs=et[q][:, base + j * 512:
                                              base + (j + 1) * 512],
                                    start=(kp == 0 and ks == 0),
                                    stop=(kp == 1 and ks == 1))

                mslot = m * 6
                if q != m // 2:
                    # clean tile: one full-2048 plain max
                    slot = mslot + (q if q < m // 2 else q - 1)
                    evac(ps, 0, QW, maxp[:, slot:slot + 1])
                else:
                    srel = _slot_rel(m)
                    if m % 2 == 0:
                        # pieces [0,1536) and [1792,2048); slot [1536,1792)
                        evac(ps, 0, 1536, maxp[:, mslot + 3:mslot + 4])
                        st = stagep.tile([128, 2048], f16, tag="st")
                        nc.scalar.copy(st[:, :256], ps[:, 1792:2048])
                        nc.vector.tensor_reduce(
                            out=maxp[:, mslot + 4:mslot + 5],
                            in_=st[:, :256], axis=Ax.X, op=Alu.max)
                    else:
                        # piece [0,1792); slot [1792,2048)
                        evac(ps, 0, 1792, maxp[:, mslot + 3:mslot + 4])
                    # slot: T = S - eqmask; max into maxp, min into minT
                    tw = twp.tile([128, W], f16, tag="tw")
                    nc.vector.tensor_tensor(
                        out=tw[:],
                        in0=ps[:, srel:srel + W],
                        in1=eqm[:, m * W:(m + 1) * W],
                        op=Alu.subtract)
                    nc.vector.tensor_reduce(
                        out=maxp[:, mslot + 5:mslot + 6], in_=tw[:],
                        axis=Ax.X, op=Alu.max)
                    nc.vector.tensor_reduce(out=minT[:, m:m + 1], in_=tw[:],
                                            axis=Ax.X, op=Alu.min)

        # ---------------- finale ----------------
        nc.vector.tensor_reduce(
            out=maxT[:],
            in_=maxp[:].rearrange("p (m s) -> p m s", m=M_TILES),
            axis=Ax.X, op=Alu.max)
        nc.vector.tensor_tensor(out=diffs[:], in0=maxT[:], in1=minT[:],
                                op=Alu.subtract)
        nc.scalar.activation(relu_d[:], diffs[:], Act.Relu, bias=negm[:],
                             accum_out=row_loss[:])
        ps1 = psum.tile([1, 1], f32, tag="ps")
        nc.tensor.matmul(ps1[:], lhsT=row_loss[:], rhs=ones_sb[:],
                         start=True, stop=True)
        nc.scalar.copy(out_sb[:], ps1[:])
        nc.sync.dma_start(out=out, in_=out_sb[:])

    nc.compile()
    return nc


class TileCtx:
    """contextmanager pairing TileContext with an ExitStack (pools close
    before the TileContext schedules)."""

    def __init__(self, nc, tile_mod):
        self.nc = nc
        self.tile_mod = tile_mod

    def __enter__(self):
        self.ctx = ExitStack()
        self.ctx.__enter__()
        self.tc = self.tile_mod.TileContext(self.nc)
        self.tc.__enter__()
        return self.tc, self.ctx

    def __exit__(self, *exc):
        self.ctx.__exit__(*exc)
        return self.tc.__exit__(*exc)


def _quantize(x):
    if USE_FP8:
        import ml_dtypes
        return x.astype(ml_dtypes.float8_e4m3)
    return x.astype(np.float16)


def _prep_inputs(embeddings, labels):
    E = np.asarray(embeddings, dtype=np.float64)
    lab = np.asarray(labels).reshape(-1)
    assert E.shape == (N, D)

    order = np.argsort(lab, kind="stable")
    E_s = E[order]
    lab_s = lab[order].astype(np.int64)
    assert np.bincount(lab_s).max() <= 64, "label multiplicity > 64"

    e = E_s / np.linalg.norm(E_s, axis=1, keepdims=True)
    e_q = _quantize(e)

    # run starts: first occurrence index of each row's label
    first_idx = np.zeros(N, dtype=np.int64)
    last_idx = np.zeros(N, dtype=np.int64)
    starts = np.r_[0, 1 + np.nonzero(np.diff(lab_s))[0]]
    ends = np.r_[starts[1:], N]
    for s, t in zip(starts, ends):
        first_idx[s:t] = s
        last_idx[s:t] = t

    in_maps = []
    for c in range(NCORES):
        # columns of each tile's positive window -> fixed slots via a
        # per-core permutation
        perm = np.full(N, -1, dtype=np.int64)
        wcols_all = []
        for m in range(M_TILES):
            g = 8 * m + c
            r0 = 128 * g
            lo = first_idx[r0]
            hi = last_idx[r0 + 127]          # exclusive
            span = hi - lo
            assert span <= W, (c, m, span)
            w0 = min(max(lo - (W - span) // 2, 0), N - W)
            # window must still contain [lo, hi)
            w0 = min(max(w0, hi - W), lo)
            slot_abs = QW * (m // 2) + _slot_rel(m)
            perm[slot_abs:slot_abs + W] = np.arange(w0, w0 + W)
            wcols_all.append(np.arange(w0, w0 + W))
        used = np.concatenate(wcols_all)
        assert len(np.unique(used)) == len(used), "overlapping windows"
        rest = np.setdiff1d(np.arange(N), used, assume_unique=False)
        perm[perm < 0] = rest
        assert len(np.unique(perm)) == N

        e_perm = e_q[perm]                    # [N, D] quantized, cols permuted
        A = np.ascontiguousarray(e_perm.T)    # [D, N]
        ET_np = np.ascontiguousarray(
            A.reshape(2, 2, 128, NQ, QW).transpose(2, 3, 0, 1, 4)
            .reshape(128, NQ * 8192))

        gsel = [8 * m + c for m in range(M_TILES)]
        blk = np.ascontiguousarray(
            e_q.reshape(64, 128, D)[gsel].reshape(1024, D))
        B = np.ascontiguousarray(blk.T)       # [D, 1024]
        BLKT_np = np.ascontiguousarray(
            B.reshape(2, 2, 128, M_TILES, 128).transpose(2, 0, 3, 1, 4)
            .reshape(128, 4096))

        # eq mask: 4.0 where window-column label == row label
        blk_lab = lab_s.reshape(64, 128)[gsel]          # [8, 128]
        eqm_np = np.zeros((128, M_TILES * W), dtype=np.float16)
        for m in range(M_TILES):
            wl = lab_s[wcols_all[m]]                     # [W]
            eqm_np[:, m * W:(m + 1) * W] = np.where(
                wl[None, :] == blk_lab[m][:, None], 4.0, 0.0)

        in_maps.append({
            "ET": ET_np,
            "BLKT": BLKT_np,
            "EQM": eqm_np,
        })
    return in_maps


def kernel(embeddings, labels):
    from concourse.bass_utils import run_bass_kernel_spmd

    in_maps = _prep_inputs(embeddings, labels)
    nc = _build_program()
    res = run_bass_kernel_spmd(nc, in_maps, core_ids=list(range(NCORES)))
    global LAST_RESULTS
    LAST_RESULTS = res
    total = sum(float(r["out"][0, 0]) for r in res.results)
    return np.float32(total / N)


LAST_RESULTS = None


# revision 18
# speedup vs baseline: 3.1866x; 1.2265x over previous
"""BatchHardTripletLoss on 8 Trainium2 NeuronCores.

Math (rows sorted by label):
  e = embeddings / ||embeddings||          (host, fp64)
  S = e @ e.T                              (device, fp8e4m3 DoubleRow matmul)
  T = S - 4 * [label_i == label_j]
  loss_row = relu(max_j T - min_j T - 3.7)  (= relu(hard_pos - hard_neg + 0.3))
  out = mean(loss_row)

min_j T always lands on a same-label element (the -4 shift beats any s >= -1);
self (s=1) is never the min unless the row has no other positive, in which
case max_j T < 0.7 keeps the relu at zero either way.

Sharding: rows sorted by label, 64 row-tiles of 128. Core c owns global
tiles g = 8m + c (m = 0..7). With sorted labels all positives of tile g
live within +-(mult-1) rows of the tile, a span <= 256 when the max label
multiplicity is <= 64. The host applies a PER-CORE column permutation to
the gathered/normalized embedding matrix so that tile m's positive window
lands at a FIXED slot: columns [2048*(m//2) + 1536, +1792) for even m,
[2048*(m//2) + 1792, +2048) for odd m. A column permutation doesn't change
row-wise max/min, so mining is: plain max over non-slot pieces, eq-masked
max/min (mask precomputed on host) over the 256-wide slot. One SPMD
program serves all cores; all data-dependent layout lives on the host.

Device pipeline per (quad q, tile m): 8 fp8 DoubleRow matmuls accumulate a
[128, 2048] f32 psum block of S. Plain pieces are max-folded to a scalar;
PSUM is only readable by the Scalar(Act) and Vector engines (GpSimd can
neither touch PSUM nor run max), so most tiles go Act psum->fp16 stage +
DVE 2x-rate fp16 folds, with every 6th tile a direct DVE psum reduce to
balance the two engines. The slot piece is one tensor_tensor_reduce
(T = S - eqmask, accum max) plus a min reduce. maxT/minT -> relu -> sum ->
one scalar DMA per core; the host adds the 8 scalars and divides by N.
"""

import os
import numpy as np
from contextlib import ExitStack

N, D = 8192, 512
NCORES = 8
M_TILES = 8          # row tiles per core
NQ = 4               # column quads
QW = 2048            # quad width
W = 256              # positive-window slot width
MARGIN_C = 3.7       # 4 - 1 + MARGIN(0.3); loss = relu(maxT - minT - 3.7)
NEG = -1.0e30

USE_FP8 = os.environ.get("KFP8", "1") == "1"   # fp8e4m3 + DoubleRow matmuls
USE_DR = os.environ.get("KDR", "1") == "1"     # DoubleRow perf mode


def _slot_rel(m):
    """Slot of tile m lives in quad m//2 at this relative offset."""
    return 1536 if m % 2 == 0 else 1792


def _build_program():
    import concourse.bass as bass
    import concourse.bacc as bacc
    import concourse.tile as tile
    from concourse import mybir

    f16 = mybir.dt.float16
    f32 = mybir.dt.float32
    f8 = mybir.dt.float8e4
    edt = f8 if USE_FP8 else f16
    Alu = mybir.AluOpType
    Act = mybir.ActivationFunctionType
    Ax = mybir.AxisListType

    nc = bacc.Bacc("TRN2", target_bir_lowering=False, debug=False,
                   num_devices=NCORES)

    # ET[p, g*8192 + kp*4096 + ks*2048 + c] = e8perm[g*2048+c, (2kp+ks)*128+p]
    ET = nc.dram_tensor("ET", [128, NQ * 8192], edt, kind="ExternalInput").ap()
    # BLKT[p, kp*2048 + m*256 + ks*128 + r] = blk[m*128+r, (2kp+ks)*128+p]
    BLKT = nc.dram_tensor("BLKT", [128, 4096], edt, kind="ExternalInput").ap()
    # EQM[p, m*256 + j] = 4.0 if label(window col j of m) == label(row p of m)
    EQM = nc.dram_tensor("EQM", [128, M_TILES * W], f16,
                         kind="ExternalInput").ap()
    out = nc.dram_tensor("out", [1, 1], f32, kind="ExternalOutput").ap()

    HW = QW // 2         # psum half-tile width (2 PSUM banks)
    NEGH = -60000.0      # fp16-safe "-inf" for the max accumulators
    PMAX = 8             # maxp partial slots per m

    with TileCtx(nc, tile) as (tc, ctx):
        persist = ctx.enter_context(tc.tile_pool(name="persist", bufs=1))
        psum = ctx.enter_context(tc.tile_pool(name="ps", bufs=4, space="PSUM"))
        stagep = ctx.enter_context(tc.tile_pool(name="stage", bufs=4))
        foldp = ctx.enter_context(tc.tile_pool(name="fold", bufs=2))
        twp = ctx.enter_context(tc.tile_pool(name="tw", bufs=2))

        et = [persist.tile([128, 8192], edt, tag=f"et{g}", name=f"et{g}")
              for g in range(NQ)]
        blkt = persist.tile([128, 4096], edt, tag="blkt")
        eqm = persist.tile([128, M_TILES * W], f16, tag="eqm")
        # per-m running fp16 max accumulator over all staged plain pieces
        acc = [persist.tile([128, 512], f16, tag=f"acc{m}", name=f"acc{m}")
               for m in range(M_TILES)]
        # per-m scalar partials: direct psum reduces + acc total + slot max
        maxp = persist.tile([128, M_TILES * PMAX], f32, tag="maxp")
        maxT = persist.tile([128, M_TILES], f32, tag="maxT")
        minT = persist.tile([128, M_TILES], f32, tag="minT")
        diffs = persist.tile([128, M_TILES], f32, tag="diffs")
        relu_d = persist.tile([128, M_TILES], f32, tag="relud")
        row_loss = persist.tile([128, 1], f32, tag="rowloss")
        ones_sb = persist.tile([128, 1], f32, tag="ones")
        negm = persist.tile([128, 1], f32, tag="negm")
        out_sb = persist.tile([1, 1], f32, tag="outsb")

        nc.sync.dma_start(out=blkt[:], in_=BLKT)
        for g in range(NQ):
            nc.sync.dma_start(out=et[g][:],
                              in_=ET[:, g * 8192:(g + 1) * 8192])
        nc.sync.dma_start(out=eqm[:], in_=EQM)

        nc.vector.memset(maxp[:], NEG)
        for m in range(M_TILES):
            nc.vector.memset(acc[m][:], NEGH)
        nc.vector.memset(ones_sb[:], 1.0)
        nc.vector.memset(negm[:], -MARGIN_C)

        clean_ctr = [0]
        vslot = [0] * M_TILES     # next free V-partial index for tile m

        def acc_max(m, src, w):
            """acc[m][:, :w] = max(acc, src) — fp16 SBUF, 2x DVE rate."""
            nc.vector.tensor_tensor(out=acc[m][:, :w], in0=acc[m][:, :w],
                                    in1=src, op=Alu.max)

        def mm_half(ps, q, m, h):
            """4 DoubleRow (or 8 plain) matmuls: S block for quad-half h."""
            for kp in range(2):
                if USE_FP8 and USE_DR:
                    lhsT = blkt[:, kp * 2048 + m * 256:
                                kp * 2048 + (m + 1) * 256].rearrange(
                                    "p (s r) -> p s r", s=2)
                    rhs4 = et[q][:, kp * 4096:(kp + 1) * 4096].rearrange(
                        "p (s c) -> p s c", s=2)
                    for j in range(2):
                        c0 = h * HW + j * 512
                        nc.tensor.matmul(
                            ps[:, j * 512:(j + 1) * 512],
                            lhsT=lhsT,
                            rhs=rhs4[:, :, c0:c0 + 512],
                            start=(kp == 0), stop=(kp == 1),
                            perf_mode=mybir.MatmulPerfMode.DoubleRow)
                else:
                    for ks in range(2):
                        lhsT = blkt[:, kp * 2048 + m * 256 + ks * 128:
                                    kp * 2048 + m * 256 + (ks + 1) * 128]
                        base = kp * 4096 + ks * 2048 + h * HW
                        for j in range(2):
                            nc.tensor.matmul(
                                ps[:, j * 512:(j + 1) * 512],
                                lhsT=lhsT,
                                rhs=et[q][:, base + j * 512:
                                          base + (j + 1) * 512],
                                start=(kp == 0 and ks == 0),
                                stop=(kp == 1 and ks == 1))

        def evac_clean(ps, m):
            """Plain-max a full [128, 1024] psum half.

            PSUM is only readable by Act and DVE (GpSimd can't touch PSUM or
            run max). Most halves: Act stages psum->fp16, DVE merges into
            acc[m] at 2x fp16 rate. Every 6th: direct DVE psum reduce."""
            route_a = clean_ctr[0] % 6 != 5
            clean_ctr[0] += 1
            if not route_a:
                s = vslot[m]
                vslot[m] += 1
                assert s < PMAX - 2
                nc.vector.tensor_reduce(
                    out=maxp[:, m * PMAX + s:m * PMAX + s + 1],
                    in_=ps[:], axis=Ax.X, op=Alu.max)
                return
            st = stagep.tile([128, HW], f16, tag="st")
            nc.scalar.copy(st[:], ps[:])
            acc_max(m, st[:, :512], 512)
            acc_max(m, st[:, 512:1024], 512)

        def evac_slot(ps, m):
            """Slot-carrying half [128, 1024]: stage whole half to fp16,
            acc-max the plain pieces, eq-mask min/max the 256-wide slot."""
            st = stagep.tile([128, HW], f16, tag="st")
            nc.scalar.copy(st[:], ps[:])
            srel = _slot_rel(m) - HW          # 512 (even m) or 768 (odd m)
            acc_max(m, st[:, :512], 512)
            plain2 = 768 if m % 2 == 0 else 512   # the other 256 plain piece
            acc_max(m, st[:, plain2:plain2 + 256], 256)
            tw = twp.tile([128, W], f16, tag="tw")
            nc.vector.tensor_tensor(out=tw[:], in0=st[:, srel:srel + W],
                                    in1=eqm[:, m * W:(m + 1) * W],
                                    op=Alu.subtract)
            nc.vector.tensor_reduce(
                out=maxp[:, m * PMAX + PMAX - 1:m * PMAX + PMAX],
                in_=tw[:], axis=Ax.X, op=Alu.max)
            nc.vector.tensor_reduce(out=minT[:, m:m + 1], in_=tw[:],
                                    axis=Ax.X, op=Alu.min)

        # ---------------- mining ----------------
        for q in range(NQ):
            for m in range(M_TILES):
                ph = []
                for h in range(2):
                    ps = psum.tile([128, HW], f32, tag="ps")
                    mm_half(ps, q, m, h)
                    ph.append(ps)
                for h in range(2):
                    if q == m // 2 and h == 1:
                        evac_slot(ph[h], m)
                    else:
                        evac_clean(ph[h], m)

        # ---------------- finale ----------------
        for m in range(M_TILES):
            ft = foldp.tile([128, 256], f16, tag="ft")
            nc.vector.tensor_tensor(out=ft[:], in0=acc[m][:, :256],
                                    in1=acc[m][:, 256:512], op=Alu.max)
            nc.vector.tensor_reduce(
                out=maxp[:, m * PMAX + PMAX - 2:m * PMAX + PMAX - 1],
                in_=ft[:], axis=Ax.X, op=Alu.max)
        nc.vector.tensor_reduce(
            out=maxT[:],
            in_=maxp[:].rearrange("p (m s) -> p m s", m=M_TILES),
            axis=Ax.X, op=Alu.max)
        nc.vector.tensor_tensor(out=diffs[:], in0=maxT[:], in1=minT[:],
                                op=Alu.subtract)
        nc.scalar.activation(relu_d[:], diffs[:], Act.Relu, bias=negm[:],
                             accum_out=row_loss[:])
        ps1 = psum.tile([1, 1], f32, tag="ps")
        nc.tensor.matmul(ps1[:], lhsT=row_loss[:], rhs=ones_sb[:],
                         start=True, stop=True)
        nc.scalar.copy(out_sb[:], ps1[:])
        nc.sync.dma_start(out=out, in_=out_sb[:])

    nc.compile()
    return nc


class TileCtx:
    """contextmanager pairing TileContext with an ExitStack (pools close
    before the TileContext schedules)."""

    def __init__(self, nc, tile_mod):
        self.nc = nc
        self.tile_mod = tile_mod

    def __enter__(self):
        self.ctx = ExitStack()
        self.ctx.__enter__()
        self.tc = self.tile_mod.TileContext(self.nc)
        self.tc.__enter__()
        return self.tc, self.ctx

    def __exit__(self, *exc):
        self.ctx.__exit__(*exc)
        return self.tc.__exit__(*exc)


def _quantize(x):
    if USE_FP8:
        import ml_dtypes
        return x.astype(ml_dtypes.float8_e4m3)
    return x.astype(np.float16)


def _prep_inputs(embeddings, labels):
    E = np.asarray(embeddings, dtype=np.float64)
    lab = np.asarray(labels).reshape(-1)
    assert E.shape == (N, D)

    order = np.argsort(lab, kind="stable")
    E_s = E[order]
    lab_s = lab[order].astype(np.int64)
    assert np.bincount(lab_s).max() <= 64, "label multiplicity > 64"

    e = E_s / np.linalg.norm(E_s, axis=1, keepdims=True)
    e_q = _quantize(e)

    # run starts: first occurrence index of each row's label
    first_idx = np.zeros(N, dtype=np.int64)
    last_idx = np.zeros(N, dtype=np.int64)
    starts = np.r_[0, 1 + np.nonzero(np.diff(lab_s))[0]]
    ends = np.r_[starts[1:], N]
    for s, t in zip(starts, ends):
        first_idx[s:t] = s
        last_idx[s:t] = t

    in_maps = []
    for c in range(NCORES):
        # columns of each tile's positive window -> fixed slots via a
        # per-core permutation
        perm = np.full(N, -1, dtype=np.int64)
        wcols_all = []
        for m in range(M_TILES):
            g = 8 * m + c
            r0 = 128 * g
            lo = first_idx[r0]
            hi = last_idx[r0 + 127]          # exclusive
            span = hi - lo
            assert span <= W, (c, m, span)
            w0 = min(max(lo - (W - span) // 2, 0), N - W)
            # window must still contain [lo, hi)
            w0 = min(max(w0, hi - W), lo)
            slot_abs = QW * (m // 2) + _slot_rel(m)
            perm[slot_abs:slot_abs + W] = np.arange(w0, w0 + W)
            wcols_all.append(np.arange(w0, w0 + W))
        used = np.concatenate(wcols_all)
        assert len(np.unique(used)) == len(used), "overlapping windows"
        rest = np.setdiff1d(np.arange(N), used, assume_unique=False)
        perm[perm < 0] = rest
        assert len(np.unique(perm)) == N

        e_perm = e_q[perm]                    # [N, D] quantized, cols permuted
        A = np.ascontiguousarray(e_perm.T)    # [D, N]
        ET_np = np.ascontiguousarray(
            A.reshape(2, 2, 128, NQ, QW).transpose(2, 3, 0, 1, 4)
            .reshape(128, NQ * 8192))

        gsel = [8 * m + c for m in range(M_TILES)]
        blk = np.ascontiguousarray(
            e_q.reshape(64, 128, D)[gsel].reshape(1024, D))
        B = np.ascontiguousarray(blk.T)       # [D, 1024]
        BLKT_np = np.ascontiguousarray(
            B.reshape(2, 2, 128, M_TILES, 128).transpose(2, 0, 3, 1, 4)
            .reshape(128, 4096))

        # eq mask: 4.0 where window-column label == row label
        blk_lab = lab_s.reshape(64, 128)[gsel]          # [8, 128]
        eqm_np = np.zeros((128, M_TILES * W), dtype=np.float16)
        for m in range(M_TILES):
            wl = lab_s[wcols_all[m]]                     # [W]
            eqm_np[:, m * W:(m + 1) * W] = np.where(
                wl[None, :] == blk_lab[m][:, None], 4.0, 0.0)

        in_maps.append({
            "ET": ET_np,
            "BLKT": BLKT_np,
            "EQM": eqm_np,
        })
    return in_maps


def kernel(embeddings, labels):
    from concourse.bass_utils import run_bass_kernel_spmd

    in_maps = _prep_inputs(embeddings, labels)
    nc = _build_program()
    res = run_bass_kernel_spmd(nc, in_maps, core_ids=list(range(NCORES)))
    global LAST_RESULTS
    LAST_RESULTS = res
    total = sum(float(r["out"][0, 0]) for r in res.results)
    return np.float32(total / N)


LAST_RESULTS = None


# revision 25
# speedup vs baseline: 3.2011x; 1.0045x over previous
"""BatchHardTripletLoss on 8 Trainium2 NeuronCores.

Math (rows sorted by label):
  e = embeddings / ||embeddings||          (host, fp64)
  S = e @ e.T                              (device, fp8e4m3 DoubleRow matmul)
  T = S - 4 * [label_i == label_j]
  loss_row = relu(max_j T - min_j T - 3.7)  (= relu(hard_pos - hard_neg + 0.3))
  out = mean(loss_row)

min_j T always lands on a same-label element (the -4 shift beats any s >= -1);
self (s=1) is never the min unless the row has no other positive, in which
case max_j T < 0.7 keeps the relu at zero either way.

Sharding: rows sorted by label, 64 row-tiles of 128. Core c owns global
tiles g = 8m + c (m = 0..7). With sorted labels all positives of tile g
live within +-(mult-1) rows of the tile, a span <= 256 when the max label
multiplicity is <= 64. The host applies a PER-CORE column permutation to
the gathered/normalized embedding matrix so that tile m's positive window
lands at a FIXED slot: columns [2048*(m//2) + 1536, +1792) for even m,
[2048*(m//2) + 1792, +2048) for odd m. A column permutation doesn't change
row-wise max/min, so mining is: plain max over non-slot pieces, eq-masked
max/min (mask precomputed on host) over the 256-wide slot. One SPMD
program serves all cores; all data-dependent layout lives on the host.

Device pipeline per (quad q, tile m): 8 fp8 DoubleRow matmuls accumulate a
[128, 2048] f32 psum block of S. Plain pieces are max-folded to a scalar;
PSUM is only readable by the Scalar(Act) and Vector engines (GpSimd can
neither touch PSUM nor run max), so most tiles go Act psum->fp16 stage +
DVE 2x-rate fp16 folds, with every 6th tile a direct DVE psum reduce to
balance the two engines. The slot piece is one tensor_tensor_reduce
(T = S - eqmask, accum max) plus a min reduce. maxT/minT -> relu -> sum ->
one scalar DMA per core; the host adds the 8 scalars and divides by N.
"""

import os
import numpy as np
from contextlib import ExitStack

N, D = 8192, 512
NCORES = 8
M_TILES = 8          # row tiles per core
NQ = 4               # column quads
QW = 2048            # quad width
W = 256              # positive-window slot width
MARGIN_C = 3.7       # 4 - 1 + MARGIN(0.3); loss = relu(maxT - minT - 3.7)
NEG = -1.0e30

USE_FP8 = os.environ.get("KFP8", "1") == "1"   # fp8e4m3 + DoubleRow matmuls
USE_DR = os.environ.get("KDR", "1") == "1"     # DoubleRow perf mode


def _slot_rel(m):
    """Slot of tile m lives in quad m//2 at this relative offset."""
    return 1536 if m % 2 == 0 else 1792


def _build_program():
    import concourse.bass as bass
    import concourse.bacc as bacc
    import concourse.tile as tile
    from concourse import mybir

    f16 = mybir.dt.float16
    f32 = mybir.dt.float32
    f8 = mybir.dt.float8e4
    edt = f8 if USE_FP8 else f16
    Alu = mybir.AluOpType
    Act = mybir.ActivationFunctionType
    Ax = mybir.AxisListType

    nc = bacc.Bacc("TRN2", target_bir_lowering=False, debug=False,
                   num_devices=NCORES)

    # ET[p, g*8192 + kp*4096 + ks*2048 + c] = e8perm[g*2048+c, (2kp+ks)*128+p]
    ET = nc.dram_tensor("ET", [128, NQ * 8192], edt, kind="ExternalInput").ap()
    # BLKT[p, kp*2048 + m*256 + ks*128 + r] = blk[m*128+r, (2kp+ks)*128+p]
    BLKT = nc.dram_tensor("BLKT", [128, 4096], edt, kind="ExternalInput").ap()
    # EQM[p, m*256 + j] = 4.0 if label(window col j of m) == label(row p of m)
    EQM = nc.dram_tensor("EQM", [128, M_TILES * W], f16,
                         kind="ExternalInput").ap()
    out = nc.dram_tensor("out", [1, 1], f32, kind="ExternalOutput").ap()

    HW = QW // 2         # psum half-tile width (2 PSUM banks)
    NEGH = -60000.0      # fp16-safe "-inf" for the max accumulators
    PMAX = 8             # maxp partial slots per m

    with TileCtx(nc, tile) as (tc, ctx):
        persist = ctx.enter_context(tc.tile_pool(name="persist", bufs=1))
        psum = ctx.enter_context(tc.tile_pool(name="ps", bufs=4, space="PSUM"))
        stagep = ctx.enter_context(tc.tile_pool(name="stage", bufs=6))
        foldp = ctx.enter_context(tc.tile_pool(name="fold", bufs=3))
        twp = ctx.enter_context(tc.tile_pool(name="tw", bufs=3))

        et = [persist.tile([128, 8192], edt, tag=f"et{g}", name=f"et{g}")
              for g in range(NQ)]
        blkt = persist.tile([128, 4096], edt, tag="blkt")
        eqm = persist.tile([128, M_TILES * W], f16, tag="eqm")
        # per-m running fp16 max accumulator over all staged plain pieces
        acc = [persist.tile([128, HW], f16, tag=f"acc{m}", name=f"acc{m}")
               for m in range(M_TILES)]
        # per-m scalar partials: direct psum reduces + acc total + slot max
        maxp = persist.tile([128, M_TILES * PMAX], f32, tag="maxp")
        maxT = persist.tile([128, M_TILES], f32, tag="maxT")
        minT = persist.tile([128, M_TILES], f32, tag="minT")
        diffs = persist.tile([128, M_TILES], f32, tag="diffs")
        relu_d = persist.tile([128, M_TILES], f32, tag="relud")
        row_loss = persist.tile([128, 1], f32, tag="rowloss")
        ones_sb = persist.tile([128, 1], f32, tag="ones")
        negm = persist.tile([128, 1], f32, tag="negm")
        out_sb = persist.tile([1, 1], f32, tag="outsb")

        # split DMAs into 2KB-line chunks so they fan out across queues and
        # the first quad lands quickly
        nc.sync.dma_start(out=blkt[:, :2048], in_=BLKT[:, :2048])
        nc.sync.dma_start(out=blkt[:, 2048:], in_=BLKT[:, 2048:])
        for g in range(NQ):
            for ch in range(4):
                nc.sync.dma_start(
                    out=et[g][:, ch * 2048:(ch + 1) * 2048],
                    in_=ET[:, g * 8192 + ch * 2048:g * 8192 + (ch + 1) * 2048])
        nc.sync.dma_start(out=eqm[:], in_=EQM)

        nc.vector.memset(maxp[:], NEG)
        for m in range(M_TILES):
            nc.vector.memset(acc[m][:], NEGH)
        nc.vector.memset(ones_sb[:], 1.0)
        nc.vector.memset(negm[:], -MARGIN_C)

        clean_ctr = [0]
        vslot = [0] * M_TILES     # next free V-partial index for tile m

        def acc_max(m, src, lo, w):
            """acc[m][:, lo:lo+w] = max(acc, src) — fp16 SBUF, 2x DVE rate."""
            nc.vector.tensor_tensor(out=acc[m][:, lo:lo + w],
                                    in0=acc[m][:, lo:lo + w],
                                    in1=src, op=Alu.max)

        def mm_half(ps, q, m, h):
            """4 DoubleRow (or 8 plain) matmuls: S block for quad-half h."""
            for kp in range(2):
                if USE_FP8 and USE_DR:
                    lhsT = blkt[:, kp * 2048 + m * 256:
                                kp * 2048 + (m + 1) * 256].rearrange(
                                    "p (s r) -> p s r", s=2)
                    rhs4 = et[q][:, kp * 4096:(kp + 1) * 4096].rearrange(
                        "p (s c) -> p s c", s=2)
                    for j in range(2):
                        c0 = h * HW + j * 512
                        nc.tensor.matmul(
                            ps[:, j * 512:(j + 1) * 512],
                            lhsT=lhsT,
                            rhs=rhs4[:, :, c0:c0 + 512],
                            start=(kp == 0), stop=(kp == 1),
                            perf_mode=mybir.MatmulPerfMode.DoubleRow)
                else:
                    for ks in range(2):
                        lhsT = blkt[:, kp * 2048 + m * 256 + ks * 128:
                                    kp * 2048 + m * 256 + (ks + 1) * 128]
                        base = kp * 4096 + ks * 2048 + h * HW
                        for j in range(2):
                            nc.tensor.matmul(
                                ps[:, j * 512:(j + 1) * 512],
                                lhsT=lhsT,
                                rhs=et[q][:, base + j * 512:
                                          base + (j + 1) * 512],
                                start=(kp == 0 and ks == 0),
                                stop=(kp == 1 and ks == 1))

        def evac_clean(ps, m):
            """Plain-max a full [128, 1024] psum half.

            PSUM is only readable by Act and DVE (GpSimd can't touch PSUM or
            run max). Most halves: Act stages psum->fp16, DVE merges into
            acc[m] at 2x fp16 rate. Every 6th: direct DVE psum reduce."""
            route_a = clean_ctr[0] % 7 != 6
            clean_ctr[0] += 1
            if not route_a:
                s = vslot[m]
                vslot[m] += 1
                assert s < PMAX - 2
                nc.vector.tensor_reduce(
                    out=maxp[:, m * PMAX + s:m * PMAX + s + 1],
                    in_=ps[:], axis=Ax.X, op=Alu.max)
                return
            st = stagep.tile([128, HW], f16, tag="st")
            nc.scalar.copy(st[:], ps[:])
            acc_max(m, st[:], 0, HW)

        def evac_slot(ps, m):
            """Slot-carrying half [128, 1024]: stage whole half to fp16,
            acc-max the plain pieces, eq-mask min/max the 256-wide slot."""
            st = stagep.tile([128, HW], f16, tag="st")
            nc.scalar.copy(st[:], ps[:])
            srel = _slot_rel(m) - HW          # 512 (even m) or 768 (odd m)
            acc_max(m, st[:, :512], 0, 512)
            plain2 = 768 if m % 2 == 0 else 512   # the other 256 plain piece
            acc_max(m, st[:, plain2:plain2 + 256], 512, 256)
            tw = twp.tile([128, W], f16, tag="tw")
            nc.vector.tensor_tensor(out=tw[:], in0=st[:, srel:srel + W],
                                    in1=eqm[:, m * W:(m + 1) * W],
                                    op=Alu.subtract)
            nc.vector.tensor_reduce(
                out=maxp[:, m * PMAX + PMAX - 1:m * PMAX + PMAX],
                in_=tw[:], axis=Ax.X, op=Alu.max)
            nc.vector.tensor_reduce(out=minT[:, m:m + 1], in_=tw[:],
                                    axis=Ax.X, op=Alu.min)

        def acc_finale(m):
            """Fold acc[m] [128, 1024] fp16 down into its maxp partial."""
            f1 = foldp.tile([128, 512], f16, tag="ff1")
            nc.vector.tensor_tensor(out=f1[:], in0=acc[m][:, :512],
                                    in1=acc[m][:, 512:1024], op=Alu.max)
            ft = foldp.tile([128, 256], f16, tag="ft")
            nc.vector.tensor_tensor(out=ft[:], in0=f1[:, :256],
                                    in1=f1[:, 256:512], op=Alu.max)
            nc.vector.tensor_reduce(
                out=maxp[:, m * PMAX + PMAX - 2:m * PMAX + PMAX - 1],
                in_=ft[:], axis=Ax.X, op=Alu.max)

        # ---------------- mining ----------------
        for q in range(NQ):
            for m in range(M_TILES):
                ph = []
                for h in range(2):
                    ps = psum.tile([128, HW], f32, tag="ps")
                    mm_half(ps, q, m, h)
                    ph.append(ps)
                for h in range(2):
                    if q == m // 2 and h == 1:
                        evac_slot(ph[h], m)
                    else:
                        evac_clean(ph[h], m)
                if q == NQ - 1:
                    # acc[m] saw its last update; fold it now so the tail
                    # overlaps with the remaining tiles' mining
                    acc_finale(m)

        # ---------------- finale ----------------
        nc.vector.tensor_reduce(
            out=maxT[:],
            in_=maxp[:].rearrange("p (m s) -> p m s", m=M_TILES),
            axis=Ax.X, op=Alu.max)
        nc.vector.tensor_tensor(out=diffs[:], in0=maxT[:], in1=minT[:],
                                op=Alu.subtract)
        nc.scalar.activation(relu_d[:], diffs[:], Act.Relu, bias=negm[:],
                             accum_out=row_loss[:])
        ps1 = psum.tile([1, 1], f32, tag="ps")
        nc.tensor.matmul(ps1[:], lhsT=row_loss[:], rhs=ones_sb[:],
                         start=True, stop=True)
        nc.scalar.copy(out_sb[:], ps1[:])
        nc.sync.dma_start(out=out, in_=out_sb[:])

    nc.compile()
    return nc


class TileCtx:
    """contextmanager pairing TileContext with an ExitStack (pools close
    before the TileContext schedules)."""

    def __init__(self, nc, tile_mod):
        self.nc = nc
        self.tile_mod = tile_mod

    def __enter__(self):
        self.ctx = ExitStack()
        self.ctx.__enter__()
        self.tc = self.tile_mod.TileContext(self.nc)
        self.tc.__enter__()
        return self.tc, self.ctx

    def __exit__(self, *exc):
        self.ctx.__exit__(*exc)
        return self.tc.__exit__(*exc)


def _quantize(x):
    if USE_FP8:
        import ml_dtypes
        return x.astype(ml_dtypes.float8_e4m3)
    return x.astype(np.float16)


def _prep_inputs(embeddings, labels):
    E = np.asarray(embeddings, dtype=np.float64)
    lab = np.asarray(labels).reshape(-1)
    assert E.shape == (N, D)

    order = np.argsort(lab, kind="stable")
    E_s = E[order]
    lab_s = lab[order].astype(np.int64)
    assert np.bincount(lab_s).max() <= 64, "label multiplicity > 64"

    e = E_s / np.linalg.norm(E_s, axis=1, keepdims=True)
    e_q = _quantize(e)

    # run starts: first occurrence index of each row's label
    first_idx = np.zeros(N, dtype=np.int64)
    last_idx = np.zeros(N, dtype=np.int64)
    starts = np.r_[0, 1 + np.nonzero(np.diff(lab_s))[0]]
    ends = np.r_[starts[1:], N]
    for s, t in zip(starts, ends):
        first_idx[s:t] = s
        last_idx[s:t] = t

    in_maps = []
    for c in range(NCORES):
        # columns of each tile's positive window -> fixed slots via a
        # per-core permutation
        perm = np.full(N, -1, dtype=np.int64)
        wcols_all = []
        for m in range(M_TILES):
            g = 8 * m + c
            r0 = 128 * g
            lo = first_idx[r0]
            hi = last_idx[r0 + 127]          # exclusive
            span = hi - lo
            assert span <= W, (c, m, span)
            w0 = min(max(lo - (W - span) // 2, 0), N - W)
            # window must still contain [lo, hi)
            w0 = min(max(w0, hi - W), lo)
            slot_abs = QW * (m // 2) + _slot_rel(m)
            perm[slot_abs:slot_abs + W] = np.arange(w0, w0 + W)
            wcols_all.append(np.arange(w0, w0 + W))
        used = np.concatenate(wcols_all)
        assert len(np.unique(used)) == len(used), "overlapping windows"
        rest = np.setdiff1d(np.arange(N), used, assume_unique=False)
        perm[perm < 0] = rest
        assert len(np.unique(perm)) == N

        e_perm = e_q[perm]                    # [N, D] quantized, cols permuted
        A = np.ascontiguousarray(e_perm.T)    # [D, N]
        ET_np = np.ascontiguousarray(
            A.reshape(2, 2, 128, NQ, QW).transpose(2, 3, 0, 1, 4)
            .reshape(128, NQ * 8192))

        gsel = [8 * m + c for m in range(M_TILES)]
        blk = np.ascontiguousarray(
            e_q.reshape(64, 128, D)[gsel].reshape(1024, D))
        B = np.ascontiguousarray(blk.T)       # [D, 1024]
        BLKT_np = np.ascontiguousarray(
            B.reshape(2, 2, 128, M_TILES, 128).transpose(2, 0, 3, 1, 4)
            .reshape(128, 4096))

        # eq mask: 4.0 where window-column label == row label
        blk_lab = lab_s.reshape(64, 128)[gsel]          # [8, 128]
        eqm_np = np.zeros((128, M_TILES * W), dtype=np.float16)
        for m in range(M_TILES):
            wl = lab_s[wcols_all[m]]                     # [W]
            eqm_np[:, m * W:(m + 1) * W] = np.where(
                wl[None, :] == blk_lab[m][:, None], 4.0, 0.0)

        in_maps.append({
            "ET": ET_np,
            "BLKT": BLKT_np,
            "EQM": eqm_np,
        })
    return in_maps


def kernel(embeddings, labels):
    from concourse.bass_utils import run_bass_kernel_spmd

    in_maps = _prep_inputs(embeddings, labels)
    nc = _build_program()
    res = run_bass_kernel_spmd(nc, in_maps, core_ids=list(range(NCORES)))
    global LAST_RESULTS
    LAST_RESULTS = res
    total = sum(float(r["out"][0, 0]) for r in res.results)
    return np.float32(total / N)


LAST_RESULTS = None
